# revision 65
# baseline (speedup 1.0000x reference)
"""Trainium2 Bass kernel for nn_DendriticANN (fp8-DoubleRow version).

Network (reference.py):
    h = BN(leaky(x @ W_in.T + b_in))                       [B, H]
    for l in range(L):
        xn   = h / max(||h||_row, 1e-12)                   row-wise L2 normalize
        dend = leaky(einsum('bi,ndi->bnd', xn, Wd[l]))     [B, H, D]
        out  = leaky(einsum('bnd,nd->bn', dend, soma[l]))  [B, H]
        h    = BN(leaky(out))
    y = h @ W_out.T + b_out                                [B, OUT]

Sharding: data-parallel over batch across 8 cores (B=2048 -> 256 rows/core),
all parameters replicated.  BatchNorm stats combined with one small bf16
AllGather per BN stage (3 total).

Key structural choices (on top of the fp16 baseline's algebra):
 - The dendritic matmuls run as fp8e4 DoubleRow (0.5 cycles/output column,
   two K=128 tiles per instruction).  Raw e4m3 is far too noisy (measured
   ~8e-2 end-to-end vs the 2e-2 gate), so both operands are hi/lo split:
       v = (Whi + Wlo) @ xhi + Whi @ xlo      (the lo*lo term is negligible)
   Three fp8 products per k-tile instead of one fp16 pass = 1.33x PE
   throughput at fp16-like accuracy (measured 3.4e-3 end-to-end).
 - leaky is positively homogeneous, so the row L2-normalization commutes with
   the dendritic stage: matmuls consume UNNORMALIZED h16 = 16*h and
   rb = 1/||h16|| is applied once per feature group after the d-reduction.
 - |soma|*32 is folded into the dendritic weight columns (anti-subnormal and
   e4m3-range placement); the sign is applied by per-tile accumulate chains.
   FOUR interleaved DVE chains hide the scalar_tensor_tensor latency (327ns
   each, no 2x mode); walrus rejects STT/TensorScalarPtr on Pool (V3 ISA),
   so Pool only carries plain tensor_tensor work (hsq, merge halves) and the
   batch-stat sums ride Act's Square/Lrelu accum_out.  The last group uses
   two chains and a narrowing quad taper so the exposed stats tail drains
   fast.
 - BN affine in S-form with amul folded into the Sqrt's scale: varq =
   Q - S^2/B; denom = sqrt((varq + B FS^2 eps)/amul^2); a = 1/denom;
   Sm = -S/B computed in parallel right after the gather-reduce.  The apply
   is u = lq + Sm (starts two hops after the reduce) then xhi = fp8(a*u)
   directly (gates the first matmul), h16 = a*u on Act for rinv, and
   xlo = fp8(h16 - xhi).  The whole boundary chain runs on DVE: cross-engine
   handoffs cost ~600-900ns of semaphore latency per hop in this regime,
   far more than DVE's 130-330ns op times.
 - The stats AllGather runs in bf16 (per-core sums are O(16)/O(300); 2^-9
   noise ~4e-5 on the mean).  The stats DMA + collective + affine chain is
   emitted under tc.high_priority() so the tile scheduler doesn't bury the
   tiny stats DMAs behind 512KB weight-chunk DMAs.
 - Weight streaming: per layer 4 streams (hi/lo x k-pair) of 8 chunk DMAs
   [128, 4096] fp8 (512KB); a chunk covers 4 PSUM quads.  Pool bufs=4 gives
   ~2.6MB prefetch per stream, enough to ride through the DMA-rate deficit
   (fp8 quads consume 512KB/1.36us ~ 400GB/s vs 358GB/s DMA).
 - The cost model (and silicon) drops the PE clock after idle gaps; dummy
   matmuls chained through the collective windows and the startup DMA wait
   keep the p-state up.  Chain lengths are sized to end just after the
   post-collective apply chain so they do not push the next stage's matmuls.

Workaround: this walrus build rejects instructions carrying more than one
sync wait ("Too many sync wait commands").  Before every compile we rewrite
the BIR JSON, moving excess waits onto same-engine NoOps inserted right
before the owning instruction.
"""

import json

import numpy as np
import ml_dtypes

import concourse.bass as bass
import concourse.mybir as mybir
import concourse.tile as tile
from concourse.bass_utils import run_bass_kernel_spmd

# ---------------------------------------------------------------- problem dims
N_CORES = 8
B, IN, H, D, OUT, L = 2048, 1024, 512, 32, 10, 2
BL = B // N_CORES            # 256 batch rows per core
ND = H * D                   # 16384 dendrite columns per layer
NG = H // 128                # 4 feature groups of 128
KD = H // 128                # 4 K-tiles of the dendritic contraction
KP = KD // 2                 # 2 DoubleRow k-pairs
KIN = IN // 128              # 8 K-tiles for the input matmul
NTILE = ND // 128            # 128 dendrite tiles per layer
BN_EPS = 1e-5
SLOPE = 0.01
FOLD_SCALE = 32.0            # |soma| fold into dendritic weights
XS = 16.0                    # activation scale into e4m3
F32 = mybir.dt.float32
F16 = mybir.dt.float16
BF16 = mybir.dt.bfloat16
F8 = mybir.dt.float8e4
E4 = ml_dtypes.float8_e4m3
MM_DT = F16

CCOLS = 4096                 # fp8 image columns per weight chunk (512KB)
NCC = (2 * ND) // CCOLS      # 8 chunk-columns per layer per stream
QPC = 4                      # quads per chunk-column
QW = 4                       # tiles per PSUM quad

WARM_BOOT = 8
WARM_N = {0: 124, 1: 118, 2: 121}   # per-stage collective-window warm lengths

# ------------------------------------------------- walrus 1-wait workaround


_patch_state = {"installed": False, "counter": 0}


def _split_excess_waits(bir_json):
    m = json.loads(bir_json)
    moved = 0
    for func in m.get("functions", []):
        for blk in func.get("blocks", []):
            new_insts = []
            for inst in blk.get("instructions", []):
                si = inst.get("sync_info") or {}
                waits = si.get("on_wait") or []
                if len(waits) > 1:
                    for w in waits[:-1]:
                        _patch_state["counter"] += 1
                        new_insts.append({
                            "opcode": "NoOp",
                            "name": f"I-waitsplit-{_patch_state['counter']}",
                            "engine": inst.get("engine", "SP"),
                            "ins": [],
                            "outs": [],
                            "debug": inst.get("debug", 0),
                            "sync_info": {"on_wait": [w], "on_update": []},
                        })
                        moved += 1
                    si["on_wait"] = [waits[-1]]
                    inst["sync_info"] = si
                new_insts.append(inst)
            blk["instructions"] = new_insts
    return json.dumps(m).encode(), moved


def _install_compile_patch():
    if _patch_state["installed"]:
        return
    _patch_state["installed"] = True
    import concourse.bass_utils as bass_utils
    import concourse.bass2jax as bass2jax

    orig = bass_utils.compile_bir_kernel

    def patched(bir_json, tmpdir, neff_name="file.neff"):
        if isinstance(bir_json, str):
            bir_json = bir_json.encode()
        bir_json, _ = _split_excess_waits(bir_json)
        return orig(bir_json, tmpdir, neff_name)

    bass_utils.compile_bir_kernel = patched
    bass2jax.compile_bir_kernel = patched


_install_compile_patch()

# ------------------------------------------------------------------ bass build


def build_nc():
    nc = bass.Bass(num_devices=N_CORES)

    xt_d = nc.dram_tensor("xt", [128, KIN * BL], MM_DT, kind="ExternalInput")
    w_in_d = nc.dram_tensor("w_in", [128, KIN * H], MM_DT,
                            kind="ExternalInput")
    b_in_d = nc.dram_tensor("b_in", [128, NG], F32, kind="ExternalInput")
    # fp8 weight images: [L, img(hi/lo), kpair, 128, 2*ND]
    wd_d = nc.dram_tensor("wd8", [L, 2, KP, 128, 2 * ND], F8,
                          kind="ExternalInput")
    sgn_d = nc.dram_tensor("sgn", [L, 128, NG * D], F32, kind="ExternalInput")
    aps_d = nc.dram_tensor("aps", [L, 128, 16], F32, kind="ExternalInput")
    w_out_d = nc.dram_tensor("w_out", [128, NG * OUT], MM_DT,
                             kind="ExternalInput")
    b_out_d = nc.dram_tensor("b_out", [OUT, 1], F32, kind="ExternalInput")
    y_d = nc.dram_tensor("y", [OUT, BL], F32, kind="ExternalOutput")

    inv_b = 1.0 / B
    rtb = float(np.sqrt(B))
    A = mybir.AluOpType
    Lrelu = mybir.ActivationFunctionType.Lrelu
    Prelu = mybir.ActivationFunctionType.Prelu
    Ident = mybir.ActivationFunctionType.Identity
    Square = mybir.ActivationFunctionType.Square
    Sqrt = mybir.ActivationFunctionType.Sqrt
    Rsqrt = mybir.ActivationFunctionType.Rsqrt
    DR = mybir.MatmulPerfMode.DoubleRow

    with tile.TileContext(nc) as tc:
        with (
            tc.tile_pool(name="const", bufs=1) as constp,
            tc.tile_pool(name="wstream", bufs=4) as wstream,
            tc.tile_pool(name="sm", bufs=4) as smp,
            tc.tile_pool(name="acts", bufs=3) as acts,
            tc.tile_pool(name="accp", bufs=2) as accp,
            tc.tile_pool(name="work", bufs=3) as work,
            tc.tile_pool(name="vec", bufs=2) as vec,
            tc.tile_pool(name="psq", bufs=3, space="PSUM") as psq,
            tc.tile_pool(name="psm", bufs=1, space="PSUM") as psm,
            tc.tile_pool(name="psd", bufs=1, space="PSUM") as psd,
            tc.tile_pool(name="dram", bufs=2 * 3, space="DRAM") as dramp,
        ):
            # PE pre-warm source (no data deps; PE ramps during startup DMAs)
            warm_src = constp.tile([128, 512], MM_DT)
            nc.vector.memset(warm_src[:], 1.0)
            ones_row = constp.tile([1, 128], MM_DT)
            nc.gpsimd.memset(ones_row[:], 1.0)
            ones_col = constp.tile([128, 1], MM_DT)
            nc.gpsimd.memset(ones_col[:], 1.0)

            ps_w0 = psd.tile([1, 512], F32, tag="ps_warm", name="warm_boot")
            for _ in range(WARM_BOOT):
                nc.tensor.matmul(ps_w0[:], warm_src[:, 0:1], warm_src[:],
                                 start=True, stop=True)

            # ---------------- constants.  w_in group-major; group 0 first so
            # the input matmuls start after ~0.75MB of DMA.
            xt_sb = constp.tile([128, KIN * BL], MM_DT)
            w_in_sb = constp.tile([128, KIN * H], MM_DT)
            nc.sync.dma_start(w_in_sb[:, 0:IN], w_in_d[:, 0:IN])
            nc.sync.dma_start(xt_sb[:], xt_d[:])
            for g in range(1, NG):
                nc.sync.dma_start(w_in_sb[:, g * IN:(g + 1) * IN],
                                  w_in_d[:, g * IN:(g + 1) * IN])
            b_in_sb = constp.tile([128, NG], F32)
            nc.sync.dma_start(b_in_sb[:], b_in_d[:])
            sgn_sb = {}
            aps_sb = {}
            for l in range(L):
                t = constp.tile([128, NG * D], F32, tag=f"sgn{l}",
                                name=f"sgn{l}")
                nc.sync.dma_start(t[:], sgn_d[l])
                sgn_sb[l] = t
                t2 = constp.tile([128, 16], F32, tag=f"aps{l}",
                                 name=f"aps{l}")
                nc.sync.dma_start(t2[:], aps_d[l])
                aps_sb[l] = t2
            w_out_sb = constp.tile([128, NG * OUT], MM_DT)
            nc.sync.dma_start(w_out_sb[:], w_out_d[:])
            b_out_sb = constp.tile([OUT, 1], F32)
            nc.sync.dma_start(b_out_sb[:], b_out_d[:])
            # sqrt scale/bias consts: s2 = 1/amul^2, bias = B*FS^2*eps*s2
            sc_t, bi_t = {}, {}
            for stage in range(L + 1):
                fs2 = 1.0 if stage == 0 else FOLD_SCALE * FOLD_SCALE
                amul = (XS if stage < L else 1.0) * rtb
                s2 = 1.0 / (amul * amul)
                t1 = constp.tile([128, 1], F32, tag=f"sc{stage}",
                                 name=f"sc{stage}")
                nc.gpsimd.memset(t1[:], s2)
                sc_t[stage] = t1
                t2 = constp.tile([128, 1], F32, tag=f"bi{stage}",
                                 name=f"bi{stage}")
                nc.gpsimd.memset(t2[:], B * fs2 * BN_EPS * s2)
                bi_t[stage] = t2

            h16_tiles = None     # [128, BL] fp16 = 16*h per k-tile
            xhi = xlo = None     # [128, KD*BL] fp8 pair images
            rb16 = None          # [128, BL] fp16 broadcast of 1/||h16|| rows
            lq_tiles = None

            def pe_warm(gate_ap, n, tag):
                """Dummy matmuls keeping the PE p-state up through a
                collective window.  First reads gate_ap (ready at window
                start); the rest chain WAW through one psum slot."""
                ps_w = psd.tile([1, 512], F32, tag="ps_warm", name=tag)
                nc.tensor.matmul(ps_w[:, :gate_ap.shape[1]], ones_col[:],
                                 gate_ap, start=True, stop=True)
                for i in range(n):
                    src = xt_sb[:, 512 * (i % 4):512 * (i % 4 + 1)]
                    nc.tensor.matmul(ps_w[:], ones_col[:], src,
                                     start=True, stop=True)

            def bn_collective(stats_sb, tag):
                """AllGather per-core stats (bf16) and reduce to global
                sums [128, 2NG] f32.  Runs under high_priority so the tiny
                stats DMAs aren't queued behind 512KB weight chunks."""
                stats_bf = vec.tile([128, 2 * NG], BF16, tag="stats_bf")
                nc.vector.tensor_scalar(stats_bf[:], stats_sb[:, :2 * NG],
                                        1.0, None, A.mult)
                st_in = dramp.tile([128, 2 * NG], BF16, tag="st_in")
                st_out = dramp.tile([N_CORES, 128, 2 * NG], BF16, tag="st_out")
                nc.sync.dma_start(st_in[:], stats_bf[:])
                nc.gpsimd.collective_compute(
                    "AllGather", A.bypass,
                    replica_groups=[list(range(N_CORES))],
                    ins=[st_in.opt()], outs=[st_out.opt()],
                )
                stats_all = vec.tile([128, N_CORES * 2 * NG], BF16,
                                     tag="stats_all")
                nc.sync.dma_start(
                    stats_all[:].rearrange("p (r c) -> p r c", r=N_CORES),
                    st_out[:].rearrange("r p c -> p r c"))
                stats_g = vec.tile([128, 2 * NG], F32, tag="stats_g")
                nc.vector.tensor_reduce(
                    stats_g[:],
                    stats_all[:].rearrange("p (r c) -> p c r", r=N_CORES),
                    mybir.AxisListType.X, A.add)
                return stats_g

            def bn_affine(stats_g, sc_ap, bi_ap):
                """a[128,NG] and Sm[128,NG] with h_rep = a*(lq + Sm).
                S-form with amul folded into the Sqrt's scale:
                  varq   = Q - S^2/B          (B * FS^2 * var)
                  denom' = sqrt((varq + B FS^2 eps) / amul^2)
                  a      = 1/denom'  = amul/sqrt(varq + B FS^2 eps)
                  Sm     = -S/B               (computed in parallel)
                Critical depth to `a`: msq, varq, sqrt, recip."""
                Sm = vec.tile([128, NG], F32, tag="bn_sm")
                nc.gpsimd.tensor_scalar(Sm[:], stats_g[:, 0:NG], -inv_b,
                                        None, A.mult)
                msq = vec.tile([128, NG], F32, tag="bn_msq")
                nc.vector.tensor_tensor(msq[:], stats_g[:, 0:NG],
                                        stats_g[:, 0:NG], A.mult)
                varq = vec.tile([128, NG], F32, tag="bn_varq")
                nc.vector.scalar_tensor_tensor(
                    varq[:], msq[:], -inv_b, stats_g[:, NG:2 * NG],
                    A.mult, A.add)
                denom = vec.tile([128, NG], F32, tag="bn_denom")
                nc.scalar.activation(denom[:], varq[:], Sqrt,
                                     bias=bi_ap, scale=sc_ap)
                a_t = vec.tile([128, NG], F32, tag="bn_a")
                nc.vector.reciprocal(a_t[:], denom[:])
                return a_t, Sm

            def rinv_chain(h16_tiles, tag):
                """rb16 [128, BL] fp16 = broadcast rows of 1/||h16||."""
                hsq = work.tile([128, NG * BL], MM_DT, tag="hsq")
                for g in range(NG):
                    nc.gpsimd.tensor_tensor(
                        hsq[:, g * BL:(g + 1) * BL], h16_tiles[g][:],
                        h16_tiles[g][:], A.mult)
                ps_r = psm.tile([1, BL], F32, tag="ps_misc")
                for g in range(NG):
                    nc.tensor.matmul(ps_r[:], ones_col[:],
                                     hsq[:, g * BL:(g + 1) * BL],
                                     start=(g == 0), stop=(g == NG - 1))
                ssq = vec.tile([1, BL], F32, tag="ssq")
                nc.vector.tensor_scalar(ssq[:], ps_r[:], 1e-24, None, A.max)
                rno = vec.tile([1, BL], F32, tag="rno")
                nc.scalar.activation(rno[:], ssq[:], Sqrt)
                rin = vec.tile([1, BL], MM_DT, tag="rin")
                with nc.allow_low_precision(reason="rinv rounding is benign"):
                    nc.vector.reciprocal(rin[:], rno[:])
                ps_b = psm.tile([128, BL], F32, tag="ps_misc")
                nc.tensor.matmul(ps_b[:], ones_row[:], rin[:],
                                 start=True, stop=True)
                rb = acts.tile([128, BL], MM_DT, tag="rb16", name=f"rb_{tag}")
                nc.scalar.activation(rb[:], ps_b[:], Ident)
                return rb

            def make_h_images(a_t, Sm, lq_tiles, stage):
                """u = lq - S/B (needs only the reduce); xhi = fp8(a*u)
                gates the first matmul; h16 = a*u (fp16, for rinv + xlo)
                runs on Act in parallel; xlo = fp8(h16 - xhi)."""
                u_tiles = []
                for g in range(NG):
                    u = work.tile([128, BL], MM_DT, tag=f"u{g}",
                                  name=f"u{stage}_{g}")
                    nc.vector.tensor_scalar(u[:], lq_tiles[g][:],
                                            Sm[:, g:g + 1], None, A.add)
                    u_tiles.append(u)
                h16 = []
                xhi_img = acts.tile([128, KD * BL], F8, tag="xhi",
                                    name=f"xhi_{stage}")
                xlo_img = acts.tile([128, KD * BL], F8, tag="xlo",
                                    name=f"xlo_{stage}")
                with nc.allow_low_precision(reason="hi/lo split is exact"):
                    for g in range(NG):
                        nc.vector.tensor_scalar(
                            xhi_img[:, g * BL:(g + 1) * BL],
                            u_tiles[g][:], a_t[:, g:g + 1], None, A.mult)
                for g in range(NG):
                    h = acts.tile([128, BL], MM_DT, tag=f"h{g}",
                                  name=f"h{stage}_{g}")
                    nc.scalar.activation(h[:], u_tiles[g][:], Ident,
                                         scale=a_t[:, g:g + 1])
                    h16.append(h)
                with nc.allow_low_precision(reason="hi/lo split is exact"):
                    for g in range(NG):
                        eng = nc.vector
                        eng.tensor_tensor(
                            xlo_img[:, g * BL:(g + 1) * BL], h16[g][:],
                            xhi_img[:, g * BL:(g + 1) * BL], A.subtract)
                return h16, xhi_img, xlo_img

            for stage in range(L + 1):
                stats_sb = vec.tile([128, 2 * NG], F32, tag="stats")
                lq_tiles = []

                if stage == 0:
                    # input layer: one PSUM quad, one quarter per group
                    ps = psq.tile([128, NG * BL], F32, tag="psq")
                    for g in range(NG):
                        for k in range(KIN):
                            nc.tensor.matmul(
                                ps[:, g * BL:(g + 1) * BL],
                                w_in_sb[:, g * IN + 128 * k:
                                        g * IN + 128 * (k + 1)],
                                xt_sb[:, k * BL:(k + 1) * BL],
                                start=(k == 0), stop=(k == KIN - 1))
                    for g in range(NG):
                        lq = acts.tile([128, BL], MM_DT, tag=f"lq{g}",
                                       name=f"lq0_{g}")
                        if g % 2 == 0:
                            nc.scalar.activation(
                                lq[:], ps[:, g * BL:(g + 1) * BL], Lrelu,
                                bias=b_in_sb[:, g:g + 1], alpha=SLOPE,
                                accum_out=stats_sb[:, g:g + 1])
                            junk = work.tile([128, BL], MM_DT, tag="junk")
                            nc.scalar.activation(
                                junk[:], lq[:], Square,
                                accum_out=stats_sb[:, NG + g:NG + g + 1])
                        else:
                            tb = work.tile([128, BL], MM_DT, tag="tb")
                            nc.vector.tensor_scalar(
                                tb[:], ps[:, g * BL:(g + 1) * BL],
                                b_in_sb[:, g:g + 1], None, A.add)
                            nc.vector.scalar_tensor_tensor(
                                lq[:], tb[:], SLOPE, tb[:], A.mult, A.max,
                                accum_out=stats_sb[:, g:g + 1])
                            junk = work.tile([128, BL], MM_DT, tag="junk")
                            nc.vector.scalar_tensor_tensor(
                                junk[:], lq[:], 1.0, lq[:], A.mult, A.mult,
                                accum_out=stats_sb[:, NG + g:NG + g + 1])
                        lq_tiles.append(lq[:])
                else:
                    l = stage - 1
                    # four interleaved accumulation chains (A/B on DVE,
                    # C/D on Pool); merged per group at the tail.
                    accA = accp.tile([128, NG * BL], MM_DT, tag="accA",
                                     name=f"accA_{l}")
                    accB = accp.tile([128, NG * BL], MM_DT, tag="accB",
                                     name=f"accB_{l}")
                    accC = accp.tile([128, NG * BL], MM_DT, tag="accC",
                                     name=f"accC_{l}")
                    accD = accp.tile([128, NG * BL], MM_DT, tag="accD",
                                     name=f"accD_{l}")
                    chains = [(accA, nc.vector), (accB, nc.vector),
                              (accC, nc.vector), (accD, nc.vector)]
                    started = set()
                    for cc in range(NCC):
                        nb = cc // 2
                        wk = {}
                        for img in range(2):
                            for p in range(KP):
                                w = wstream.tile([128, CCOLS], F8,
                                                 tag=f"w{img}{p}")
                                nc.sync.dma_start(
                                    w[:], wd_d[l, img, p][
                                        :, CCOLS * cc:CCOLS * (cc + 1)])
                                wk[(img, p)] = w
                        widths = ([QW] * QPC if cc < NCC - 1
                                  else [QW, QW, 2, 2, 2, 2])
                        tbase = 0
                        for q, qw in enumerate(widths):
                            ps = psq.tile([128, QW * BL], F32, tag="psq")
                            for j in range(qw):
                                slot = tbase + j          # tile slot in chunk
                                co = 256 * slot
                                out = ps[:, BL * j:BL * (j + 1)]
                                seq = [(0, 0, xhi), (0, 1, xhi),
                                       (1, 0, xhi), (1, 1, xhi),
                                       (0, 0, xlo), (0, 1, xlo)]
                                for i, (img, p, xim) in enumerate(seq):
                                    wt = wk[(img, p)][:, co:co + 256] \
                                        .rearrange("a (two m) -> a two m",
                                                   two=2)
                                    xp = xim[:, 2 * p * BL:2 * (p + 1) * BL] \
                                        .rearrange("a (two b) -> a two b",
                                                   two=2)
                                    nc.tensor.matmul(
                                        out, wt, xp, start=(i == 0),
                                        stop=(i == len(seq) - 1),
                                        perf_mode=DR)
                            sm = smp.tile([128, QW * BL], MM_DT, tag="sm")
                            taper2 = (cc == NCC - 1 and qw == 2)
                            if taper2:
                                # sign pre-negated into W for these tiles;
                                # sgn*leaky(v) == Prelu_{a}(s*v') with
                                # per-partition (a, s) from the aps image
                                for j in range(qw):
                                    sl = tbase + j - 8
                                    nc.scalar.activation(
                                        sm[:, BL * j:BL * (j + 1)],
                                        ps[:, BL * j:BL * (j + 1)], Prelu,
                                        alpha=aps_sb[l][:, sl:sl + 1],
                                        scale=aps_sb[l][:, 8 + sl:9 + sl])
                            else:
                                nc.scalar.activation(sm[:, :qw * BL],
                                                     ps[:, :qw * BL], Prelu,
                                                     alpha=SLOPE)
                            if cc == NCC - 1 and q == 3:
                                sm_last = sm
                            for j in range(qw):
                                d = 16 * (cc % 2) + tbase + j
                                nch = 2 if nb == NG - 1 else 4
                                acc, eng = chains[(tbase + j) % nch]
                                accs = acc[:, nb * BL:(nb + 1) * BL]
                                sms = sm[:, BL * j:BL * (j + 1)]
                                sc = sgn_sb[l][:, nb * D + d:nb * D + d + 1]
                                key = (nb, id(acc))
                                if taper2:
                                    if key not in started:
                                        started.add(key)
                                        eng.tensor_scalar(
                                            accs, sms, 1.0, None, A.mult)
                                    else:
                                        eng.tensor_tensor(
                                            accs, accs, sms, A.add)
                                elif key not in started:
                                    started.add(key)
                                    eng.tensor_scalar(
                                        accs, sms, sc, None, A.mult)
                                else:
                                    eng.scalar_tensor_tensor(
                                        accs, sms, sc, accs, A.mult, A.add)
                            tbase += qw
                        if cc % 2 == 1:
                            # group nb finished: merge the four chains and
                            # finish all-DVE/Pool so nothing queues behind
                            # the wide Prelus on Act.
                            g = nb
                            asum = work.tile([128, BL], MM_DT, tag="asum")
                            if g == NG - 1:
                                nc.vector.tensor_tensor(
                                    asum[:], accA[:, g * BL:(g + 1) * BL],
                                    accB[:, g * BL:(g + 1) * BL], A.add)
                            else:
                                asum1 = work.tile([128, BL], MM_DT,
                                                  tag="asum1")
                                nc.vector.tensor_tensor(
                                    asum1[:], accA[:, g * BL:(g + 1) * BL],
                                    accC[:, g * BL:(g + 1) * BL], A.add)
                                asum2 = work.tile([128, BL], MM_DT,
                                                  tag="asum2")
                                nc.gpsimd.tensor_tensor(
                                    asum2[:], accB[:, g * BL:(g + 1) * BL],
                                    accD[:, g * BL:(g + 1) * BL], A.add)
                                nc.vector.tensor_tensor(
                                    asum[:], asum1[:], asum2[:], A.add)
                            lq = acts.tile([128, BL], MM_DT, tag=f"lq{g}",
                                           name=f"lq{l}_{g}")
                            if g == NG - 1:
                                # exposed tail: rinv multiply commutes inside
                                # the double-leaky max (rb > 0):
                                # lq = max(a^2*asum*rb, asum*rb)
                                u = work.tile([128, BL], MM_DT, tag="m")
                                nc.vector.tensor_tensor(u[:], asum[:],
                                                        rb16[:], A.mult)
                                nc.vector.scalar_tensor_tensor(
                                    lq[:], u[:], SLOPE * SLOPE, u[:],
                                    A.mult, A.max,
                                    accum_out=stats_sb[:, g:g + 1])
                            else:
                                m = work.tile([128, BL], MM_DT, tag="m")
                                nc.scalar.activation(m[:], asum[:], Prelu,
                                                     alpha=SLOPE * SLOPE)
                                nc.vector.scalar_tensor_tensor(
                                    lq[:], m[:], 1.0, rb16[:], A.mult, A.mult,
                                    accum_out=stats_sb[:, g:g + 1])
                            junk = work.tile([128, BL], MM_DT, tag="junk")
                            if g == NG - 1:
                                nc.vector.scalar_tensor_tensor(
                                    junk[:], lq[:], 1.0, lq[:], A.mult,
                                    A.mult,
                                    accum_out=stats_sb[:, NG + g:NG + g + 1])
                            else:
                                nc.scalar.activation(
                                    junk[:], lq[:], Square,
                                    accum_out=stats_sb[:, NG + g:NG + g + 1])
                            lq_tiles.append(lq)

                # ---- collective + affine (PE held warm through the window)
                if stage == 0:
                    pe_warm(lq_tiles[0][:], WARM_N[0], "warm0")
                else:
                    pe_warm(sm_last[:, :512], WARM_N[stage], f"warm{stage}")
                with tc.high_priority():
                    stats_g = bn_collective(stats_sb, f"s{stage}")
                    a_t, Sm = bn_affine(stats_g, sc_t[stage][:],
                                        bi_t[stage][:])

                if stage < L:
                    h16_tiles, xhi, xlo = make_h_images(a_t, Sm, lq_tiles,
                                                        stage)
                    rb16 = rinv_chain(h16_tiles, f"s{stage}")
                else:
                    # fold BN affine into W_out: y = sum_g (w_g * a_g)^T lq_g
                    #                                 + W^T b + b_out
                    tb16 = vec.tile([128, NG], MM_DT, tag="tb16")
                    nc.vector.tensor_tensor(tb16[:], Sm[:], a_t[:], A.mult)
                    wos = work.tile([128, NG * OUT], MM_DT, tag="wos")
                    for g in range(NG):
                        nc.vector.tensor_scalar(
                            wos[:, g * OUT:(g + 1) * OUT],
                            w_out_sb[:, g * OUT:(g + 1) * OUT],
                            a_t[:, g:g + 1], None, A.mult)
                    ps_b10 = psm.tile([OUT, 1], F32, tag="ps_misc")
                    for g in range(NG):
                        nc.tensor.matmul(ps_b10[:],
                                         w_out_sb[:, g * OUT:(g + 1) * OUT],
                                         tb16[:, g:g + 1],
                                         start=(g == 0), stop=(g == NG - 1))
                    bprime = vec.tile([OUT, 1], F32, tag="bprime")
                    nc.scalar.activation(bprime[:], ps_b10[:], Ident,
                                         bias=b_out_sb[:])
                    ps_y = psm.tile([OUT, BL], F32, tag="ps_misc")
                    for g in range(NG):
                        nc.tensor.matmul(ps_y[:],
                                         wos[:, g * OUT:(g + 1) * OUT],
                                         lq_tiles[g][:],
                                         start=(g == 0), stop=(g == NG - 1))
                    y_sb = work.tile([OUT, BL], F32, tag="y_sb")
                    nc.scalar.activation(y_sb[:], ps_y[:], Ident,
                                         bias=bprime[:])
                    nc.sync.dma_start(y_d[:], y_sb[:])

    return nc


# ------------------------------------------------------------------ host side

_cache = {}


def _get_nc():
    if "nc" not in _cache:
        _cache["nc"] = build_nc()
    return _cache["nc"]


def make_in_maps(x, W_in, b_in, Wd, soma, W_out, b_out):
    mm_np = np.float16
    xT = x.T.astype(mm_np)                                   # [IN, B]
    # w_in group-major: [1024, 512] -> [128, NG*1024], col = g*1024 + k*128
    w_in_t = np.ascontiguousarray(
        W_in.T.astype(mm_np).reshape(KIN, 128, NG, 128).transpose(
            1, 2, 0, 3).reshape(128, KIN * H))
    b_in_t = np.ascontiguousarray(
        b_in.reshape(NG, 128).T.astype(np.float32))          # [128, NG]
    # Fold |soma| * FOLD into the dendritic weight columns; sign applied by
    # the accumulate chains.  Column order: nb*4096 + d*128 + n.
    soma_c = np.abs(soma) * FOLD_SCALE                       # [L, H, D]
    fold = soma_c.transpose(0, 2, 1)[:, None, :, :]          # [L, 1, D, H]
    wd_f = Wd.transpose(0, 3, 2, 1) * fold                   # [L, i, D, H=n]
    wd_g = wd_f.reshape(L, H, D, NG, 128).transpose(0, 1, 3, 2, 4)
    wd_g = wd_g.reshape(L, H, ND)                            # [L, i, col]
    # last 8 tiles (group 3, d=24..31): fold the soma sign into the weight
    # columns; sgn*leaky is then a reverse-Prelu with per-partition
    # alpha/scale (see kernel taper path)
    s3 = np.where(soma[:, 3 * 128:, 24:32] >= 0, 1.0,
                  -1.0).astype(np.float32)                   # [L, 128n, 8d]
    colsgn = s3.transpose(0, 2, 1).reshape(L, 1, 8 * 128)    # [L, 1, (d n)]
    wd_g[:, :, ND - 1024:] *= colsgn
    aps = np.empty((L, 128, 16), np.float32)
    aps[:, :, 0:8] = np.where(s3 > 0, SLOPE, 1.0 / SLOPE)
    aps[:, :, 8:16] = np.where(s3 > 0, 1.0, SLOPE)
    aps = np.ascontiguousarray(aps)
    # rows (i) into (kpair, slot); cols into (tile, m):
    # image[l, p, r, 256*tile + 128*slot + m] = wd_g[l, 128*(2p+s)+r, 128*t+m]
    wd_k = wd_g.reshape(L, KP, 2, 128, NTILE, 128)           # [L,p,s,r,t,m]
    wd_p = wd_k.transpose(0, 1, 3, 4, 2, 5).reshape(L, KP, 128, 2 * ND)
    wd_p32 = np.clip(wd_p, -240.0, 240.0).astype(np.float32)
    hi = wd_p32.astype(E4)
    lo = np.clip(wd_p32 - hi.astype(np.float32), -240.0, 240.0).astype(E4)
    wd8 = np.ascontiguousarray(np.stack([hi, lo], axis=1))   # [L,2,KP,128,2ND]
    sgn = np.where(soma >= 0, 1.0, -1.0).astype(np.float32)  # [L, H, D]
    sgn2 = np.ascontiguousarray(
        sgn.reshape(L, NG, 128, D).transpose(0, 2, 1, 3).reshape(
            L, 128, NG * D))
    w_out_t = np.ascontiguousarray(
        W_out.T.astype(mm_np).reshape(NG, 128, OUT).transpose(1, 0, 2).reshape(
            128, NG * OUT))
    common = dict(
        w_in=w_in_t,
        b_in=b_in_t,
        wd8=wd8,
        aps=aps,
        sgn=sgn2,
        w_out=w_out_t,
        b_out=np.ascontiguousarray(b_out.reshape(OUT, 1), dtype=np.float32),
    )
    in_maps = []
    for c in range(N_CORES):
        m = dict(common)
        xs = xT[:, BL * c:BL * (c + 1)]                      # [IN, BL]
        m["xt"] = np.ascontiguousarray(
            xs.reshape(KIN, 128, BL).transpose(1, 0, 2).reshape(128, KIN * BL))
        in_maps.append(m)
    return in_maps


def kernel(x, W_in, b_in, Wd, soma, W_out, b_out):
    in_maps = make_in_maps(np.asarray(x, dtype=np.float32),
                           np.asarray(W_in), np.asarray(b_in),
                           np.asarray(Wd), np.asarray(soma),
                           np.asarray(W_out), np.asarray(b_out))
    nc = _get_nc()
    try:
        res = run_bass_kernel_spmd(nc, in_maps, core_ids=list(range(N_CORES)))
    except Exception:
        # transient device state (e.g. NRT_EXEC_UNIT_UNRECOVERABLE) -- retry
        # once with a core reset requested
        import os
        os.environ.setdefault("NEURON_RT_RESET_CORES", "1")
        res = run_bass_kernel_spmd(nc, in_maps, core_ids=list(range(N_CORES)))
    y = np.concatenate([r["y"] for r in res.results], axis=1)  # [OUT, B]
    return np.ascontiguousarray(y.T, dtype=np.float32)


if __name__ == "__main__":
    rng = np.random.default_rng(0)
    x = rng.standard_normal((B, IN), dtype=np.float32)
    W_in = (rng.standard_normal((H, IN), dtype=np.float32) / np.sqrt(IN))
    b_in_a = np.zeros(H, np.float32)
    Wd_a = rng.standard_normal((L, H, D, H), dtype=np.float32) * 0.1
    soma_a = rng.standard_normal((L, H, D), dtype=np.float32) * 0.1
    W_out = rng.standard_normal((OUT, H), dtype=np.float32) / np.sqrt(H)
    b_out_a = np.zeros(OUT, np.float32)
    y = kernel(x=x, W_in=W_in, b_in=b_in_a, Wd=Wd_a, soma=soma_a,
               W_out=W_out, b_out=b_out_a)
    print("kernel output:", y.shape, y.dtype, float(np.abs(y).max()))
